# revision 1
# baseline (speedup 1.0000x reference)
"""Trainium2 Bass kernel for nn_CrossAttentionBlock.

Reference computation (B=16384, C=1024, D=128):
    g_x     = x0 @ g_w.T + g_b          # [B, D]
    theta_x = x1 @ theta_w.T + theta_b  # [B, D]
    phi_x   = x1 @ phi_w.T + phi_b      # [B, D]
    f[b,i,j] = phi_x[b,i] * theta_x[b,j]
    attn = softmax(f, axis=-1)
    y[b,i] = sum_j attn[b,i,j] * g_x[b,j]
    out = y @ W_w.T + W_b + x0          # [B, C]

Unnormalized form used on-chip (no max-subtraction needed: |f| <= ~40, exp
fits fp32/bf16 comfortably):
    E_T[j,i] = exp(theta[b,j] * phi[b,i])        (per b, j on partitions)
    num[i] = sum_j g[b,j] * E_T[j,i]   den[i] = sum_j E_T[j,i]
    y[b,i] = num[i] / den[i]

Sharding: pure data parallel over batch across 8 cores (2048 rows/core).

Per-core pipeline:
  P1: theta/phi projections -> [b,d] fp16 tiles; g projection -> g_xT [d,b]
      interleaved with ones into g1 [d, 2b] bf16.
  P2: per-b rank-1 outer-product matmuls (K=1, lhsT=theta-row, rhs=phi-row,
      4-way row-tiled at partitions {0,32,64,96} via a realignment DMA) write
      f_T [j,i] into PSUM; grouped ACT exp (PSUM->SBUF, bf16) produces E_T;
      per-b reduce matmuls (lhsT=E_T_b, rhs=[g|1]) accumulate num/den in
      PSUM; DVE reciprocal+mul produce y_T [d,b] bf16.
  P3: final matmul (lhsT=y_T group, rhs=W_w.T, N=1024) + residual add + DMA.
"""

import os
from contextlib import ExitStack

import numpy as np

import concourse.bass as bass
import concourse.tile as tile
from concourse import bacc
from concourse import mybir

F32 = mybir.dt.float32
F16 = mybir.dt.float16
BF16 = mybir.dt.bfloat16

NCORES = 8
B, C, D = 16384, 1024, 128
KC = C // 128  # 8 contraction chunks for the projections

# batch rows per f/E tile in the attention phase (1024 fp32 = 2 PSUM banks)
FTILE = 8


def build_bass(bc: int, reps: int = 1):
    """Build the per-core bass program for a batch slice of `bc` rows.

    reps>1 repeats the whole pipeline (for (T_R - T_1)/(R-1) timing)."""
    ng = bc // 128  # groups of 128 rows
    nq = max(1, bc // 512)  # 512-row groups for the g projection
    qsz = min(bc, 512)
    n_ftiles = (bc + FTILE - 1) // FTILE

    nc = bacc.Bacc(trn_type="TRN2")

    x1t = nc.dram_tensor("x1t", [C, bc], F16, kind="ExternalInput")
    x0t = nc.dram_tensor("x0t", [C, bc], F16, kind="ExternalInput")
    x0r = nc.dram_tensor("x0r", [bc, C], F16, kind="ExternalInput")
    wc = nc.dram_tensor("wc", [C, 2 * D], F16, kind="ExternalInput")
    gwt = nc.dram_tensor("gwt", [C, D], F16, kind="ExternalInput")
    wwt = nc.dram_tensor("wwt", [D, C], BF16, kind="ExternalInput")
    btp = nc.dram_tensor("btp", [128, 2 * D], F32, kind="ExternalInput")
    gb = nc.dram_tensor("gb", [D, 1], F32, kind="ExternalInput")
    out = nc.dram_tensor("out", [bc, C], F32, kind="ExternalOutput")

    with tile.TileContext(nc) as tc, ExitStack() as ctx:
        singles = ctx.enter_context(tc.tile_pool(name="singles", bufs=1))

        # ---- static weights / constants in SBUF ----
        wc_sb = singles.tile([128, KC, 2 * D], F16)  # [c-part, chunk, 256]
        nc.sync.dma_start(wc_sb, wc[:, :].rearrange("(k p) d -> p k d", p=128))
        gwt_sb = singles.tile([128, KC, D], F16)
        nc.sync.dma_start(gwt_sb, gwt[:, :].rearrange("(k p) d -> p k d", p=128))
        wwt_sb = singles.tile([128, C], BF16)
        nc.sync.dma_start(wwt_sb, wwt[:, :])
        btp_sb = singles.tile([128, 2 * D], F32)
        nc.sync.dma_start(btp_sb, btp[:, :])
        gb_sb = singles.tile([128, 1], F32)
        nc.sync.dma_start(gb_sb, gb[:, :])

        # persistent per-core activations
        tp16 = singles.tile([128, ng, 2 * D], F16)  # [theta|phi] fp16, [b-part, G, 256]
        g1 = singles.tile([128, 2 * bc], BF16)  # g_xT interleaved with ones [d, 2b]
        y16 = singles.tile([128, bc], BF16)  # y_T [d, b] bf16
        nc.vector.memset(g1, 1.0)

        from contextlib import nullcontext
        rep_ctx = tc.For_i(0, reps, 1) if reps > 1 else nullcontext()
        with rep_ctx:
            # ===== P1 interleaved into P2: per-group projections =====
            with (
                tc.tile_pool(name="xin", bufs=3) as xin,
                tc.tile_pool(name="xg", bufs=2) as xg,
                tc.tile_pool(name="projpsum", bufs=1, space="PSUM") as projpsum,
                tc.tile_pool(name="fpsum", bufs=2, space="PSUM") as fpsum,
                tc.tile_pool(name="ndpsum", bufs=1, space="PSUM") as ndpsum,
                tc.tile_pool(name="opsum", bufs=2, space="PSUM") as opsum,
                tc.tile_pool(name="epool", bufs=3) as epool,
                tc.tile_pool(name="ndsb", bufs=2) as ndsb,
                tc.tile_pool(name="rec", bufs=2) as rec,
                tc.tile_pool(name="resid", bufs=6) as resid,
                tc.tile_pool(name="osb", bufs=3) as osb,
            ):
                g1v = g1.rearrange("p (b two) -> p b two", two=2)

                x1_tiles = [None] * ng
                x0_tiles = [None] * nq

                def emit_x1_dma(G):
                    x1_tiles[G] = xin.tile([128, KC, 128], F16, tag="xin", name="xint")
                    nc.sync.dma_start(
                        x1_tiles[G],
                        x1t[:, G * 128 : (G + 1) * 128].rearrange(
                            "(k p) b -> p k b", p=128
                        ),
                    )

                def emit_x0_dma(q):
                    x0_tiles[q] = xg.tile([128, KC, qsz], F16, tag="xg", name="xgt")
                    nc.sync.dma_start(
                        x0_tiles[q],
                        x0t[:, q * qsz : (q + 1) * qsz].rearrange(
                            "(k p) b -> p k b", p=128
                        ),
                    )

                def emit_proj_tp(G):
                    # theta/phi projection for one 128-row group
                    pt = projpsum.tile([128, 512], F32, tag="pp", name="ppt")
                    xt = x1_tiles[G]
                    for k in range(KC):
                        nc.tensor.matmul(
                            pt[:, : 2 * D], lhsT=xt[:, k, :], rhs=wc_sb[:, k, :],
                            start=(k == 0), stop=(k == KC - 1),
                        )
                    nc.vector.tensor_add(tp16[:, G, :], pt[:, : 2 * D], btp_sb)

                def emit_proj_g(q):
                    gp = projpsum.tile([128, 512], F32, tag="pp", name="gpt")
                    gp = gp[:, :qsz]
                    xt = x0_tiles[q]
                    for k in range(KC):
                        nc.tensor.matmul(
                            gp, lhsT=gwt_sb[:, k, :], rhs=xt[:, k, :],
                            start=(k == 0), stop=(k == KC - 1),
                        )
                    nc.vector.tensor_scalar_add(
                        g1v[:, q * qsz : (q + 1) * qsz, 0], gp, gb_sb
                    )

                # slot s -> (group G, b-local): natural order. Theta/phi rows
                # are realigned onto partitions 0-3 (b mod 4 -> partition), with
                # phi embedded block-diagonally in a zero-padded buffer, so one
                # K=4 matmul at tile_position (0,0) computes 4 outer products.
                # (Concurrent row-tiled positions crash the exec unit on this HW.)
                QB = 4  # batch rows per outer matmul
                QROWS = 64  # b-rows per realigned buffer quarter
                nquart = (bc + QROWS - 1) // QROWS
                QT = QROWS // QB  # quads per quarter

                f_tiles = [None] * n_ftiles
                e_tiles = [None] * n_ftiles
                nd_tiles = [None] * ng
                xr_tiles = [None] * ng

                gpq = max(1, qsz // 128)  # groups per g-projection block

                # ping-pong persistent realign buffers (zeros are static)
                thbuf = [
                    singles.tile([128, QT * D], F16, name=f"thbuf{i}")
                    for i in range(2)
                ]
                phbuf = [
                    singles.tile([128, QT * QB * D], F16, name=f"phbuf{i}")
                    for i in range(2)
                ]
                for i in range(2):
                    nc.vector.memset(phbuf[i], 0.0)

                def emit_realign(q):
                    # rows [q*QROWS, (q+1)*QROWS): theta row (QB*t+p) to
                    # (partition p, offset t*128); phi row to (partition p,
                    # offset t*512 + p*128) inside the zeroed buffer.
                    Gq, blq = divmod(q * QROWS, 128)
                    tb, pb = thbuf[q % 2], phbuf[q % 2]
                    for p in range(QB):
                        src_t = tp16[:, Gq, 0:D][blq + p : blq + QROWS : QB, :]
                        dst_t = tb[p : p + 1, :].rearrange("o (t e) -> o t e", e=D)
                        nc.gpsimd.dma_start(dst_t, src_t)
                        src_p = tp16[:, Gq, D : 2 * D][blq + p : blq + QROWS : QB, :]
                        dst_p = pb[p : p + 1, :].rearrange(
                            "o (t f) -> o t f", f=QB * D
                        )[:, :, p * D : (p + 1) * D]
                        nc.gpsimd.dma_start(dst_p, src_p)

                def emit_outers(T):
                    lo, hi = T * FTILE, min((T + 1) * FTILE, bc)
                    f_tiles[T] = fpsum.tile([128, FTILE * 128], F32, tag="f", name="ftile")
                    for s in range(lo, hi, QB):
                        G, bl = divmod(s, 128)
                        if bl == 0:
                            # prefetch input DMAs and run projections one
                            # group ahead so the PE never stalls on loads.
                            if G == 0:
                                for Gp in range(min(3, ng)):
                                    emit_x1_dma(Gp)
                                emit_x0_dma(0)
                                for Gp in range(min(2, ng)):
                                    emit_proj_tp(Gp)
                            else:
                                if G + 2 < ng:
                                    emit_x1_dma(G + 2)
                                if G + 1 < ng:
                                    emit_proj_tp(G + 1)
                            if (G + 1) % gpq == 0 and (G + 1) // gpq < nq:
                                emit_x0_dma((G + 1) // gpq)
                            if G % gpq == 0 and G > 0:
                                emit_proj_g(G // gpq)
                            nd_tiles[G] = ndpsum.tile([128, 2 * 128], F32, tag="nd", name="ndt")
                            xr_tiles[G] = resid.tile([128, C], F16, tag="xr", name="xrt")
                            nc.sync.dma_start(
                                xr_tiles[G], x0r[G * 128 : (G + 1) * 128, :]
                            )
                        q, r = divmod(s, QROWS)
                        if r == 0:
                            if q == 0:
                                emit_realign(0)
                            if q + 1 < nquart:
                                emit_realign(q + 1)
                        t = r // QB  # quad index within quarter
                        j = s - lo
                        nc.tensor.matmul(
                            f_tiles[T][:, j * 128 : (j + QB) * 128],
                            lhsT=thbuf[q % 2][0:QB, t * D : (t + 1) * D],
                            rhs=phbuf[q % 2][0:QB, t * QB * D : (t + 1) * QB * D],
                        )

                def emit_exp(T):
                    lo, hi = T * FTILE, min((T + 1) * FTILE, bc)
                    n = hi - lo
                    e_tiles[T] = epool.tile([128, FTILE * 128], BF16, tag="e", name="etile")
                    nc.scalar.activation(
                        e_tiles[T][:, : n * 128],
                        f_tiles[T][:, : n * 128],
                        mybir.ActivationFunctionType.Exp,
                    )
                    if os.environ.get("K_DOUBLE_EXP"):
                        e2 = epool.tile([128, FTILE * 128], BF16, tag="e2", name="e2tile")
                        nc.scalar.activation(
                            e2[:, : n * 128],
                            f_tiles[T][:, : n * 128],
                            mybir.ActivationFunctionType.Exp,
                        )

                def emit_reduces(T):
                    lo, hi = T * FTILE, min((T + 1) * FTILE, bc)
                    nrep = 2 if os.environ.get("K_DOUBLE_REDUCE") else 1
                    for s in range(lo, hi):
                        G, bl = divmod(s, 128)
                        j = s - lo
                        for _ in range(nrep):
                            nc.tensor.matmul(
                                nd_tiles[G][:, 2 * bl : 2 * bl + 2],
                                lhsT=e_tiles[T][:, j * 128 : (j + 1) * 128],
                                rhs=g1[:, 2 * (G * 128 + bl) : 2 * (G * 128 + bl) + 2],
                            )

                def emit_final(G):
                    ot = osb.tile([128, C], F32, tag="ot", name="ott")
                    for h in range(2):
                        op = opsum.tile([128, 512], F32, tag="op", name="opt")
                        nc.tensor.matmul(
                            op,
                            lhsT=y16[:, G * 128 : (G + 1) * 128],
                            rhs=wwt_sb[:, h * 512 : (h + 1) * 512],
                        )
                        nc.vector.tensor_add(
                            ot[:, h * 512 : (h + 1) * 512],
                            op,
                            xr_tiles[G][:, h * 512 : (h + 1) * 512],
                        )
                    nc.sync.dma_start(out[G * 128 : (G + 1) * 128, :], ot)

                def emit_divide(G):
                    nd = ndsb.tile([128, 256], F32, tag="ndsb")
                    nc.vector.tensor_copy(nd, nd_tiles[G])
                    ndv = nd.rearrange("p (b two) -> p b two", two=2)
                    r = rec.tile([128, 128], F32, tag="rec")
                    nc.vector.reciprocal(r, ndv[:, :, 1])
                    nc.vector.tensor_mul(
                        y16[:, G * 128 : (G + 1) * 128], ndv[:, :, 0], r
                    )

                # software-pipelined emission: outers(T), exp(T-1), reduces(T-2)
                for T in range(n_ftiles + 2):
                    if T < n_ftiles:
                        emit_outers(T)
                    if T == 1:
                        emit_proj_g(0)  # x0 DMA was issued at T=0; MMs here
                    if 1 <= T <= n_ftiles:
                        emit_exp(T - 1)
                    if T >= 2:
                        Tr = T - 2
                        emit_reduces(Tr)
                        # divide+final for any group fully reduced by tile Tr
                        hi = min((Tr + 1) * FTILE, bc)
                        lo = Tr * FTILE
                        for G in range(lo // 128, (hi + 127) // 128):
                            if lo < (G + 1) * 128 <= hi:
                                emit_divide(G)
                                emit_final(G)
                        if hi == bc and bc % 128 != 0:
                            emit_divide(bc // 128)
                            emit_final(bc // 128)

    nc.compile()
    return nc


_BASS_CACHE = {}


def _get_bass(bc):
    if bc not in _BASS_CACHE:
        _BASS_CACHE[bc] = build_bass(bc)
    return _BASS_CACHE[bc]


def make_core_inputs(x0, x1, g_w, g_b, theta_w, theta_b, phi_w, phi_b, W_w, W_b,
                     bc=None, ncores=NCORES):
    """Host-side preprocessing -> list of per-core input dicts."""
    n = x0.shape[0] if bc is None else bc * ncores
    bc = n // ncores

    x0 = np.asarray(x0, np.float32)[:n]
    x1 = np.asarray(x1, np.float32)[:n]
    x1t = np.ascontiguousarray(x1.T.astype(np.float16))
    x0t = np.ascontiguousarray(x0.T.astype(np.float16))
    x0r = x0 if not np.any(W_b) else (x0 + np.asarray(W_b, np.float32)[None, :])
    x0r = np.ascontiguousarray(x0r, dtype=np.float16)

    wc = np.ascontiguousarray(
        np.concatenate([np.asarray(theta_w).T, np.asarray(phi_w).T], axis=1).astype(np.float16)
    )  # [C, 2D]
    gwt = np.ascontiguousarray(np.asarray(g_w).T.astype(np.float16))  # [C, D]
    import ml_dtypes
    wwt = np.ascontiguousarray(np.asarray(W_w).T.astype(ml_dtypes.bfloat16))  # [D, C]
    btp = np.ascontiguousarray(
        np.tile(np.concatenate([np.asarray(theta_b), np.asarray(phi_b)])[None, :], (128, 1)).astype(np.float32)
    )
    gbc = np.ascontiguousarray(np.asarray(g_b, np.float32).reshape(D, 1))

    in_maps = []
    for c in range(ncores):
        sl = slice(c * bc, (c + 1) * bc)
        in_maps.append(
            {
                "x1t": np.ascontiguousarray(x1t[:, sl]),
                "x0t": np.ascontiguousarray(x0t[:, sl]),
                "x0r": np.ascontiguousarray(x0r[sl]),
                "wc": wc,
                "gwt": gwt,
                "wwt": wwt,
                "btp": btp,
                "gb": gbc,
            }
        )
    return in_maps, bc


def kernel(x0, x1, g_w, g_b, theta_w, theta_b, phi_w, phi_b, W_w, W_b):
    from concourse.bass_utils import run_bass_kernel_spmd

    in_maps, bc = make_core_inputs(
        x0, x1, g_w, g_b, theta_w, theta_b, phi_w, phi_b, W_w, W_b
    )
    nc = _get_bass(bc)
    res = run_bass_kernel_spmd(nc, in_maps, core_ids=list(range(NCORES)))
    outs = [r["out"] for r in res.results]
    return np.ascontiguousarray(np.concatenate(outs, axis=0), dtype=np.float32)



# revision 14
# speedup vs baseline: 1.4152x; 1.4152x over previous
"""Trainium2 Bass kernel for nn_CrossAttentionBlock (basis-approximation version).

Reference computation (B=16384, C=1024, D=128):
    g_x     = x0 @ g_w.T + g_b          # [B, D]
    theta_x = x1 @ theta_w.T + theta_b  # [B, D]
    phi_x   = x1 @ phi_w.T + phi_b      # [B, D]
    f[b,i,j] = phi_x[b,i] * theta_x[b,j]
    attn = softmax(f, axis=-1)
    y[b,i] = sum_j attn[b,i,j] * g_x[b,j]
    out = y @ W_w.T + W_b + x0          # [B, C]

Key identity: y[b,i] = Y_b(phi[b,i]) where Y_b(p) = sum_j g_j e^{p th_j} /
sum_j e^{p th_j} is a smooth scalar function per row b.  Instead of the
O(D^2) exp per row, evaluate Y_b exactly at L=32 grid points (chebyshev-free:
uniform p_l in [-1,1] of the per-row phi range), least-squares fit a tanh
radial basis (NB=32 units incl. a near-linear and a bias unit), and evaluate
the fitted expansion at the 128 phi targets.  exp count per row: L*D instead
of D*D (4x), and every matmul uses small-P or static weights (no per-row
128-column LDWEIGHTS).

Per-core phases (data parallel over batch, 2048 rows/core):
  P1: projections. theta_T [d,b] and g_T [d,b] via static-weight matmuls;
      phi [b,i] per group; hw_b = max_i |phi_bi| via fused abs_max reduce;
      phi_hat = phi/hw (fp16), theta_hat_T = theta_T * hw (broadcast via
      ones-outer matmul of the DMA-transposed hw row).
  P2: grid. Per grid node l: ACT computes E_l = exp(p_l * theta_hat_T) in one
      FD=2048 instruction (scale immediate); DVE forms gE_l; PE reduces
      num/den with a ones[128,1] stationary column into psum rows (32r+l,
      b//4) keyed by residue r = b%4 (stride-4 rhs APs).
  P2b: ygrid = num * recip(den); 4 static block-masked fit matmuls produce
      the per-row basis coefficients directly in the block-diagonal layout
      the eval matmul wants.
  P3: eval. Per 8-quad batch: args = coefT(5x128 static) @ qbuf (realigned
      phi_hat quads + ones row) -> tanh (ACT) -> per-quad matmul with
      lhsT=E2 (bf16, FWL) and rhs=c columns -> y_T [i,b] in psum.  Final
      y @ W_w.T + x0 as in the direct kernel.
"""

import os
from contextlib import ExitStack, nullcontext

import numpy as np

import concourse.bass as bass
import concourse.tile as tile
from concourse import bacc
from concourse import mybir

F32 = mybir.dt.float32
F16 = mybir.dt.float16
BF16 = mybir.dt.bfloat16

NCORES = 8
B, C, D = 16384, 1024, 128
KC = C // 128  # 8 contraction chunks for the projections

L = 32   # grid points
NB = 32  # basis units (30 tanh + linear + bias)
BETA = 12.0
LAM = 1e-3


def _basis_params():
    nodes = np.linspace(-1.0, 1.0, L)
    cents = np.concatenate([np.linspace(-1.05, 1.05, NB - 2), [0.0, -1.5]])
    betas = np.concatenate([np.full(NB - 2, BETA), [0.1, 50.0]])
    return nodes, cents, betas


def _fit_matrix():
    """F [NB, L]: ridge-LS fit from L grid samples to NB tanh-unit coeffs."""
    nodes, cents, betas = _basis_params()
    Bm = np.tanh(betas[None, :] * (nodes[:, None] - cents[None, :]))  # [L, NB]
    F = np.linalg.solve(Bm.T @ Bm + LAM * np.eye(NB), Bm.T)  # [NB, L]
    return F


def build_bass(bc: int):
    ng = bc // 128          # 128-row groups
    nch = bc // 512         # 512-col chunks
    nq = bc // 4            # quads
    nodes, cents, betas = _basis_params()

    nc = bacc.Bacc(trn_type="TRN2")

    x1t = nc.dram_tensor("x1t", [C, bc], F16, kind="ExternalInput")
    x0t = nc.dram_tensor("x0t", [C, bc], F16, kind="ExternalInput")
    x0r = nc.dram_tensor("x0r", [bc, C], F16, kind="ExternalInput")
    thwt = nc.dram_tensor("thwt", [C, D], F16, kind="ExternalInput")
    phwt = nc.dram_tensor("phwt", [C, D], F16, kind="ExternalInput")
    gwt = nc.dram_tensor("gwt", [C, D], F16, kind="ExternalInput")
    wwt = nc.dram_tensor("wwt", [D, C], BF16, kind="ExternalInput")
    thb = nc.dram_tensor("thb", [D, 1], F32, kind="ExternalInput")
    gb = nc.dram_tensor("gb", [D, 1], F32, kind="ExternalInput")
    phb = nc.dram_tensor("phb", [128, D], F32, kind="ExternalInput")
    fmat = nc.dram_tensor("fmat", [4 * 128, 128], F32, kind="ExternalInput")
    coeft = nc.dram_tensor("coeft", [5, 128], F16, kind="ExternalInput")
    hwdram = nc.dram_tensor("hwdram", [bc], F32, kind="Internal")
    out = nc.dram_tensor("out", [bc, C], F32, kind="ExternalOutput")

    with tile.TileContext(nc) as tc, ExitStack() as ctx:
        singles = ctx.enter_context(tc.tile_pool(name="singles", bufs=1))

        # ---- static weights / constants ----
        thwt_sb = singles.tile([128, KC, D], F16)
        nc.sync.dma_start(thwt_sb, thwt[:, :].rearrange("(k p) d -> p k d", p=128))
        phwt_sb = singles.tile([128, KC, D], F16)
        nc.sync.dma_start(phwt_sb, phwt[:, :].rearrange("(k p) d -> p k d", p=128))
        gwt_sb = singles.tile([128, KC, D], F16)
        nc.sync.dma_start(gwt_sb, gwt[:, :].rearrange("(k p) d -> p k d", p=128))
        wwt_sb = singles.tile([128, C], BF16)
        nc.sync.dma_start(wwt_sb, wwt[:, :])
        thb_sb = singles.tile([128, 1], F32)
        nc.sync.dma_start(thb_sb, thb[:, :])
        gb_sb = singles.tile([128, 1], F32)
        nc.sync.dma_start(gb_sb, gb[:, :])
        phb_sb = singles.tile([128, D], F32)
        nc.sync.dma_start(phb_sb, phb[:, :])
        fm_sb = singles.tile([128, 4, 128], F32)
        nc.sync.dma_start(fm_sb, fmat[:, :].rearrange("(r p) m -> p r m", p=128))
        coeft_sb = singles.tile([5, 128], F16)
        nc.sync.dma_start(coeft_sb, coeft[:, :])

        # sliding-window one-hot lhsT for grid reduces: col 63 ones, rest 0.
        # id127[:, 63-j : 127-j] is [128, 64] with ones in column j only.
        id127 = singles.tile([128, 127], BF16)
        nc.vector.memset(id127, 0.0)
        nc.vector.memset(id127[:, 63:64], 1.0)
        ones_row32 = singles.tile([1, 128], F32)  # hw broadcast lhsT
        nc.vector.memset(ones_row32, 1.0)

        # ---- persistent activations ----
        x1t_sb = singles.tile([128, KC, bc], F16)
        g16 = singles.tile([128, bc], BF16)        # g_T [d, b]
        thT_sb = singles.tile([128, bc], F32)      # theta_T + bias
        that32 = singles.tile([128, bc], F32)      # theta_hat_T
        phsb = singles.tile([128, ng, 128], F32)   # phi [b, G, i]
        phi16 = singles.tile([128, ng, 128], F16)  # phi_hat fp16
        hw = singles.tile([128, ng], F32)
        ihw = singles.tile([128, ng], F32)
        hwrow = singles.tile([1, bc], F32)
        hwbc = singles.tile([128, bc], F32)
        ygrid = singles.tile([128, bc // 4], F32)
        rden = singles.tile([128, bc // 4], F32)
        csb = singles.tile([128, bc], BF16)        # coeffs, col 4q+r
        y16 = singles.tile([128, bc], BF16)        # y_T [i, b]
        qbufs = [singles.tile([5, 32, 128], F16, name=f"qbuf{i}") for i in range(2)]
        for qb in qbufs:
            # rows 0-3 are overwritten by realign DMAs; row 4 stays ones
            nc.vector.memset(qb, 1.0)

        # ================= P1: projections =================
        with (
            tc.tile_pool(name="x0in", bufs=2) as x0in,
            tc.tile_pool(name="projps", bufs=2, space="PSUM") as projps,
            tc.tile_pool(name="phps", bufs=2, space="PSUM") as phps,
            tc.tile_pool(name="scr", bufs=2) as scr,
        ):
            nc.sync.dma_start(
                x1t_sb, x1t[:, :].rearrange("(k p) b -> p k b", p=128)
            )
            # theta_T per 512-chunk
            for ch in range(nch):
                sl = slice(ch * 512, (ch + 1) * 512)
                pt = projps.tile([128, 512], F32, tag="pp")
                for k in range(KC):
                    nc.tensor.matmul(
                        pt, lhsT=thwt_sb[:, k, :], rhs=x1t_sb[:, k, sl],
                        start=(k == 0), stop=(k == KC - 1),
                    )
                nc.vector.tensor_scalar_add(thT_sb[:, sl], pt, thb_sb)
            # g_T per 512-chunk
            for ch in range(nch):
                sl = slice(ch * 512, (ch + 1) * 512)
                xg = x0in.tile([128, KC, 512], F16, tag="xg")
                nc.sync.dma_start(
                    xg, x0t[:, sl].rearrange("(k p) b -> p k b", p=128)
                )
                gp = projps.tile([128, 512], F32, tag="pp")
                for k in range(KC):
                    nc.tensor.matmul(
                        gp, lhsT=gwt_sb[:, k, :], rhs=xg[:, k, :],
                        start=(k == 0), stop=(k == KC - 1),
                    )
                nc.vector.tensor_scalar_add(g16[:, sl], gp, gb_sb)
            # phi per 128-group: [b, i]
            for G in range(ng):
                sl = slice(G * 128, (G + 1) * 128)
                pp = phps.tile([128, 128], F32, tag="php")
                for k in range(KC):
                    nc.tensor.matmul(
                        pp, lhsT=x1t_sb[:, k, sl], rhs=phwt_sb[:, k, :],
                        start=(k == 0), stop=(k == KC - 1),
                    )
                nc.vector.tensor_add(phsb[:, G, :], pp, phb_sb)
                st = scr.tile([128, 1], F32, tag="st")
                nc.vector.tensor_reduce(
                    st, phsb[:, G, :], axis=mybir.AxisListType.X,
                    op=mybir.AluOpType.max, apply_absolute_value=True,
                )
                nc.vector.tensor_scalar_add(hw[:, G : G + 1], st, 1e-6)
            nc.vector.reciprocal(ihw, hw)
            for G in range(ng):
                nc.vector.tensor_scalar_mul(
                    phi16[:, G, :], phsb[:, G, :], ihw[:, G : G + 1]
                )
            # hw broadcast: [128, ng] -> (DRAM round trip) -> [1, bc]
            # -> ones-outer matmul -> [128, bc]
            nc.sync.dma_start(hwdram[:].rearrange("(p g) -> p g", p=128), hw)
            nc.sync.dma_start(
                hwrow.rearrange("o (g p) -> o g p", p=128),
                hwdram[:].rearrange("(p g) -> g p", p=128),
            )
            for ch in range(nch):
                sl = slice(ch * 512, (ch + 1) * 512)
                hb = projps.tile([128, 512], F32, tag="pp")
                nc.tensor.matmul(hb, lhsT=ones_row32, rhs=hwrow[:, sl])
                nc.vector.tensor_mul(that32[:, sl], thT_sb[:, sl], hb)

        # ================= P2: grid =================
        with tc.tile_pool(name="gridpsum", bufs=1, space="PSUM") as gridpsum:
            numps = gridpsum.tile([128, bc // 4], F32)
            denps = gridpsum.tile([128, bc // 4], F32)
            with (
                tc.tile_pool(name="epool", bufs=2) as epool,
                tc.tile_pool(name="gepool", bufs=2) as gepool,
            ):
                for li in range(L):
                    E = epool.tile([128, bc], BF16, tag="e")
                    nc.scalar.activation(
                        E, that32, mybir.ActivationFunctionType.Exp,
                        scale=float(nodes[li]),
                    )
                    gE = gepool.tile([128, bc], BF16, tag="ge")
                    nc.vector.tensor_mul(gE, g16, E)
                    Ev = E.rearrange("p (c r) -> p c r", r=4)
                    gEv = gE.rearrange("p (c r) -> p c r", r=4)
                    for r in range(4):
                        j = 32 * (r % 2) + li  # column within the 64-block
                        oh = id127[:, 63 - j : 127 - j]  # ones in column j
                        base = 64 * (r // 2)
                        st = li == 0 and r % 2 == 0
                        sp = li == L - 1 and r % 2 == 1
                        nc.tensor.matmul(
                            denps[base : base + 64, :], lhsT=oh,
                            rhs=Ev[:, :, r], start=st, stop=sp,
                        )
                        nc.tensor.matmul(
                            numps[base : base + 64, :], lhsT=oh,
                            rhs=gEv[:, :, r], start=st, stop=sp,
                        )

            # ================= P2b: ygrid + fit =================
            with tc.tile_pool(name="cps", bufs=4, space="PSUM") as cpsp:
                nc.vector.reciprocal(rden, denps)
                nc.vector.tensor_mul(ygrid, numps, rden)
                csv = csb.rearrange("p (q r) -> p q r", r=4)
                for r in range(4):
                    cp = cpsp.tile([128, bc // 4], F32, tag="cp")
                    nc.tensor.matmul(cp, lhsT=fm_sb[:, r, :], rhs=ygrid)
                    nc.vector.tensor_copy(csv[:, :, r], cp)

        # ================= P3: eval + final =================
        NQB = 8  # quads per batch
        nbatch = nq // NQB
        with (
            tc.tile_pool(name="argps", bufs=2, space="PSUM") as argps,
            tc.tile_pool(name="e2pool", bufs=3) as e2pool,
            tc.tile_pool(name="ypsp", bufs=2, space="PSUM") as ypsp,
            tc.tile_pool(name="finps", bufs=2, space="PSUM") as finps,
            tc.tile_pool(name="resid", bufs=4) as resid,
            tc.tile_pool(name="osb", bufs=3) as osb,
        ):
            xr_tiles = [None] * ng
            yps_tiles = [None] * ng

            def emit_realign(G):
                qb = qbufs[G % 2]
                for p in range(4):
                    eng = nc.gpsimd if p % 2 == 0 else nc.sync
                    eng.dma_start(
                        qb[p : p + 1, :, :], phi16[p : 128 : 4, G, :]
                    )

            def emit_xr(G):
                xr_tiles[G] = resid.tile([128, C], F16, tag="xr", name="xrt")
                nc.sync.dma_start(xr_tiles[G], x0r[G * 128 : (G + 1) * 128, :])

            def emit_final(G):
                yv = y16[:, G * 128 : (G + 1) * 128]
                ot = osb.tile([128, C], F32, tag="ot")
                for h in range(2):
                    fp = finps.tile([128, 512], F32, tag="fp")
                    nc.tensor.matmul(
                        fp, lhsT=yv, rhs=wwt_sb[:, h * 512 : (h + 1) * 512]
                    )
                    nc.vector.tensor_add(
                        ot[:, h * 512 : (h + 1) * 512], fp,
                        xr_tiles[G][:, h * 512 : (h + 1) * 512],
                    )
                nc.sync.dma_start(out[G * 128 : (G + 1) * 128, :], ot)

            emit_realign(0)
            emit_xr(0)
            emit_xr(1)
            for t in range(nbatch):
                G, tq = divmod(t * NQB, 32)  # group, quad offset within group
                if tq == 0:
                    if G + 1 < ng:
                        emit_realign(G + 1)
                    if G + 2 < ng:
                        emit_xr(G + 2)
                    yps_tiles[G] = ypsp.tile([128, 128], F32, tag="yp", name="ypt")
                qb = qbufs[G % 2]
                ap = argps.tile([128, NQB * 128], F32, tag="ap")
                for h in range(2):
                    nc.tensor.matmul(
                        ap[:, h * 512 : (h + 1) * 512],
                        lhsT=coeft_sb,
                        rhs=qb[:, tq + h * 4 : tq + (h + 1) * 4, :].rearrange(
                            "p q i -> p (q i)"
                        ),
                    )
                e2 = e2pool.tile([128, NQB * 128], BF16, tag="e2")
                nc.scalar.activation(e2, ap, mybir.ActivationFunctionType.Tanh)
                for j in range(NQB):
                    q = t * NQB + j  # global quad
                    nc.tensor.matmul(
                        yps_tiles[G][:, (tq + j) * 4 : (tq + j) * 4 + 4],
                        lhsT=e2[:, j * 128 : (j + 1) * 128],
                        rhs=csb[:, 4 * q : 4 * q + 4],
                    )
                if tq + NQB == 32:  # group complete
                    nc.vector.tensor_copy(
                        y16[:, G * 128 : (G + 1) * 128], yps_tiles[G]
                    )
                    emit_final(G)

    nc.compile()
    return nc


_BASS_CACHE = {}


def _get_bass(bc):
    if bc not in _BASS_CACHE:
        _BASS_CACHE[bc] = build_bass(bc)
    return _BASS_CACHE[bc]


def make_core_inputs(x0, x1, g_w, g_b, theta_w, theta_b, phi_w, phi_b, W_w, W_b,
                     bc=None, ncores=NCORES):
    import ml_dtypes

    n = x0.shape[0] if bc is None else bc * ncores
    bc = n // ncores

    x0 = np.asarray(x0, np.float32)[:n]
    x1 = np.asarray(x1, np.float32)[:n]
    x1t = np.ascontiguousarray(x1.T.astype(np.float16))
    x0t = np.ascontiguousarray(x0.T.astype(np.float16))
    x0r = x0 if not np.any(W_b) else (x0 + np.asarray(W_b, np.float32)[None, :])
    x0r = np.ascontiguousarray(x0r, dtype=np.float16)

    thwt = np.ascontiguousarray(np.asarray(theta_w).T.astype(np.float16))
    phwt = np.ascontiguousarray(np.asarray(phi_w).T.astype(np.float16))
    gwt = np.ascontiguousarray(np.asarray(g_w).T.astype(np.float16))
    wwt = np.ascontiguousarray(np.asarray(W_w).T.astype(ml_dtypes.bfloat16))
    thbc = np.ascontiguousarray(np.asarray(theta_b, np.float32).reshape(D, 1))
    gbc = np.ascontiguousarray(np.asarray(g_b, np.float32).reshape(D, 1))
    phbc = np.ascontiguousarray(
        np.tile(np.asarray(phi_b, np.float32)[None, :], (128, 1))
    )

    nodes, cents, betas = _basis_params()
    F = _fit_matrix()  # [NB, L]
    # fmat[r]: [128, 128] lhsT, [(r2,l), (r3,m)] = F[m, l] iff r2==r3==r
    fmat = np.zeros((4, 128, 128), np.float32)
    for r in range(4):
        fmat[r, 32 * r : 32 * r + L, 32 * r : 32 * r + NB] = F.T
    fmat = np.ascontiguousarray(fmat.reshape(4 * 128, 128).astype(np.float32))
    # coeft [5, 128]: col (32r+l): row r = betas[l], row 4 = -betas[l]*cents[l]
    coeft = np.zeros((5, 128), np.float32)
    for r in range(4):
        for l in range(NB):
            coeft[r, 32 * r + l] = betas[l]
            coeft[4, 32 * r + l] = -betas[l] * cents[l]
    # note: col (32r+l) row 4 shared across r -> -betas*cents placed per col
    coeft = np.ascontiguousarray(coeft.astype(np.float16))

    in_maps = []
    for c in range(ncores):
        sl = slice(c * bc, (c + 1) * bc)
        in_maps.append(
            {
                "x1t": np.ascontiguousarray(x1t[:, sl]),
                "x0t": np.ascontiguousarray(x0t[:, sl]),
                "x0r": np.ascontiguousarray(x0r[sl]),
                "thwt": thwt,
                "phwt": phwt,
                "gwt": gwt,
                "wwt": wwt,
                "thb": thbc,
                "gb": gbc,
                "phb": phbc,
                "fmat": fmat,
                "coeft": coeft,
            }
        )
    return in_maps, bc


def kernel(x0, x1, g_w, g_b, theta_w, theta_b, phi_w, phi_b, W_w, W_b):
    from concourse.bass_utils import run_bass_kernel_spmd

    in_maps, bc = make_core_inputs(
        x0, x1, g_w, g_b, theta_w, theta_b, phi_w, phi_b, W_w, W_b
    )
    nc = _get_bass(bc)
    res = run_bass_kernel_spmd(nc, in_maps, core_ids=list(range(NCORES)))
    outs = [r["out"] for r in res.results]
    return np.ascontiguousarray(np.concatenate(outs, axis=0), dtype=np.float32)


# revision 20
# speedup vs baseline: 2.0020x; 1.4146x over previous
"""Trainium2 Bass kernel for nn_CrossAttentionBlock (basis-approximation version).

Reference computation (B=16384, C=1024, D=128):
    g_x     = x0 @ g_w.T + g_b          # [B, D]
    theta_x = x1 @ theta_w.T + theta_b  # [B, D]
    phi_x   = x1 @ phi_w.T + phi_b      # [B, D]
    f[b,i,j] = phi_x[b,i] * theta_x[b,j]
    attn = softmax(f, axis=-1)
    y[b,i] = sum_j attn[b,i,j] * g_x[b,j]
    out = y @ W_w.T + W_b + x0          # [B, C]

Key identity: y[b,i] = Y_b(phi[b,i]) where Y_b(p) = sum_j g_j e^{p th_j} /
sum_j e^{p th_j} is a smooth scalar function per row b.  Instead of the
O(D^2) exp per row, evaluate Y_b exactly at L=32 grid points (chebyshev-free:
uniform p_l in [-1,1] of the per-row phi range), least-squares fit a tanh
radial basis (NB=32 units incl. a near-linear and a bias unit), and evaluate
the fitted expansion at the 128 phi targets.  exp count per row: L*D instead
of D*D (4x), and every matmul uses small-P or static weights (no per-row
128-column LDWEIGHTS).

Per-core phases (data parallel over batch, 2048 rows/core):
  P1: projections. theta_T [d,b] and g_T [d,b] via static-weight matmuls;
      phi [b,i] per group; hw_b = max_i |phi_bi| via fused abs_max reduce;
      phi_hat = phi/hw (fp16), theta_hat_T = theta_T * hw (broadcast via
      ones-outer matmul of the DMA-transposed hw row).
  P2: grid. Per grid node l: ACT computes E_l = exp(p_l * theta_hat_T) in one
      FD=2048 instruction (scale immediate); DVE forms gE_l; PE reduces
      num/den with a ones[128,1] stationary column into psum rows (32r+l,
      b//4) keyed by residue r = b%4 (stride-4 rhs APs).
  P2b: ygrid = num * recip(den); 4 static block-masked fit matmuls produce
      the per-row basis coefficients directly in the block-diagonal layout
      the eval matmul wants.
  P3: eval. Per 8-quad batch: args = coefT(5x128 static) @ qbuf (realigned
      phi_hat quads + ones row) -> tanh (ACT) -> per-quad matmul with
      lhsT=E2 (bf16, FWL) and rhs=c columns -> y_T [i,b] in psum.  Final
      y @ W_w.T + x0 as in the direct kernel.
"""

import os
from contextlib import ExitStack, nullcontext

import numpy as np

import concourse.bass as bass
import concourse.tile as tile
from concourse import bacc
from concourse import mybir

F32 = mybir.dt.float32
F16 = mybir.dt.float16
BF16 = mybir.dt.bfloat16

NCORES = 8
B, C, D = 16384, 1024, 128
KC = C // 128  # 8 contraction chunks for the projections

L = 24   # grid points
NB = 32  # basis units (30 tanh + linear + bias)
BETA = 12.0
LAM = 1e-3


def _basis_params():
    nodes = np.linspace(-1.0, 1.0, L)
    cents = np.concatenate([np.linspace(-1.05, 1.05, NB - 2), [0.0, -1.5]])
    betas = np.concatenate([np.full(NB - 2, BETA), [0.1, 50.0]])
    return nodes, cents, betas


def _fit_matrix():
    """F [NB, L]: ridge-LS fit from L grid samples to NB tanh-unit coeffs."""
    nodes, cents, betas = _basis_params()
    Bm = np.tanh(betas[None, :] * (nodes[:, None] - cents[None, :]))  # [L, NB]
    F = np.linalg.solve(Bm.T @ Bm + LAM * np.eye(NB), Bm.T)  # [NB, L]
    return F


def build_bass(bc: int):
    ng = bc // 128          # 128-row groups
    nch = bc // 512         # 512-col chunks
    nq = bc // 4            # quads
    nodes, cents, betas = _basis_params()

    nc = bacc.Bacc(trn_type="TRN2")

    x1t = nc.dram_tensor("x1t", [C, bc], F16, kind="ExternalInput")
    x0t = nc.dram_tensor("x0t", [C, bc], F16, kind="ExternalInput")
    x0r = nc.dram_tensor("x0r", [bc, C], F16, kind="ExternalInput")
    thwt = nc.dram_tensor("thwt", [C, D], F16, kind="ExternalInput")
    phwt = nc.dram_tensor("phwt", [C, D], F16, kind="ExternalInput")
    gwt = nc.dram_tensor("gwt", [C, D], F16, kind="ExternalInput")
    wwt = nc.dram_tensor("wwt", [D, C], BF16, kind="ExternalInput")
    thb = nc.dram_tensor("thb", [D, 1], F32, kind="ExternalInput")
    gb = nc.dram_tensor("gb", [D, 1], F32, kind="ExternalInput")
    phb = nc.dram_tensor("phb", [128, D], F32, kind="ExternalInput")
    fmat = nc.dram_tensor("fmat", [4 * 128, 128], F32, kind="ExternalInput")
    coeft = nc.dram_tensor("coeft", [5, 128], F16, kind="ExternalInput")
    hwdram = nc.dram_tensor("hwdram", [bc], F32, kind="Internal")
    out = nc.dram_tensor("out", [bc, C], F32, kind="ExternalOutput")

    with tile.TileContext(nc) as tc, ExitStack() as ctx:
        singles = ctx.enter_context(tc.tile_pool(name="singles", bufs=1))

        # ---- static weights / constants ----
        thwt_sb = singles.tile([128, KC, D], F16)
        nc.sync.dma_start(thwt_sb, thwt[:, :].rearrange("(k p) d -> p k d", p=128))
        phwt_sb = singles.tile([128, KC, D], F16)
        nc.sync.dma_start(phwt_sb, phwt[:, :].rearrange("(k p) d -> p k d", p=128))
        gwt_sb = singles.tile([128, KC, D], F16)
        nc.sync.dma_start(gwt_sb, gwt[:, :].rearrange("(k p) d -> p k d", p=128))
        wwt_sb = singles.tile([128, C], BF16)
        nc.sync.dma_start(wwt_sb, wwt[:, :])
        thb_sb = singles.tile([128, 1], F32)
        nc.sync.dma_start(thb_sb, thb[:, :])
        gb_sb = singles.tile([128, 1], F32)
        nc.sync.dma_start(gb_sb, gb[:, :])
        phb_sb = singles.tile([128, D], F32)
        nc.sync.dma_start(phb_sb, phb[:, :])
        fm_sb = singles.tile([128, 4, 128], F32)
        nc.sync.dma_start(fm_sb, fmat[:, :].rearrange("(r p) m -> p r m", p=128))
        coeft_sb = singles.tile([5, 128], F16)
        nc.sync.dma_start(coeft_sb, coeft[:, :])

        # sliding-window one-hot lhsT for grid reduces: col 63 ones, rest 0.
        # id127[:, 63-j : 127-j] is [128, 64] with ones in column j only.
        id127 = singles.tile([128, 127], BF16)
        nc.vector.memset(id127, 0.0)
        nc.vector.memset(id127[:, 63:64], 1.0)
        ones_row32 = singles.tile([1, 128], F32)  # hw broadcast lhsT
        nc.vector.memset(ones_row32, 1.0)

        # ---- persistent activations ----
        x1t_sb = singles.tile([128, KC, bc], F16)
        g16 = singles.tile([128, bc], BF16)        # g_T [d, b]
        thT_sb = singles.tile([128, bc], F32)      # theta_T + bias
        that32 = singles.tile([128, bc], F32)      # theta_hat_T
        phsb = singles.tile([128, ng, 128], F32)   # phi [b, G, i]
        phi16 = singles.tile([128, ng, 128], F16)  # phi_hat fp16
        hw = singles.tile([128, ng], F32)
        ihw = singles.tile([128, ng], F32)
        hwrow = singles.tile([1, bc], F32)
        hwbc = singles.tile([128, bc], F32)
        ygrid = singles.tile([128, bc // 4], F32)
        rden = singles.tile([128, bc // 4], F32)
        dpre = singles.tile([128, bc // 4], F32)
        csb = singles.tile([128, bc], BF16)        # coeffs, col 4q+r
        y16 = singles.tile([128, bc], BF16)        # y_T [i, b]
        qbufs = [singles.tile([5, 32, 2, 128], F16, name=f"qbuf{i}") for i in range(2)]
        for qb in qbufs:
            # rows 0-3 are overwritten by realign DMAs; row 4 stays ones
            nc.vector.memset(qb, 1.0)

        # ================= P1: projections =================
        with (
            tc.tile_pool(name="x0in", bufs=2) as x0in,
            tc.tile_pool(name="projps", bufs=2, space="PSUM") as projps,
            tc.tile_pool(name="phps", bufs=2, space="PSUM") as phps,
            tc.tile_pool(name="scr", bufs=2) as scr,
        ):
            # chunked x1t load; phi-projections first (hw chain is on the
            # critical path to the grid phase), theta per chunk after.
            for ch in range(nch):
                sl = slice(ch * 512, (ch + 1) * 512)
                nc.sync.dma_start(
                    x1t_sb[:, :, sl],
                    x1t[:, sl].rearrange("(k p) b -> p k b", p=128),
                )
                for Gs in range(4):
                    G = ch * 4 + Gs
                    gl = slice(G * 128, (G + 1) * 128)
                    pp = phps.tile([128, 128], F32, tag="php")
                    for k in range(KC):
                        nc.tensor.matmul(
                            pp, lhsT=x1t_sb[:, k, gl], rhs=phwt_sb[:, k, :],
                            start=(k == 0), stop=(k == KC - 1),
                        )
                    nc.vector.tensor_add(phsb[:, G, :], pp, phb_sb)
                    st = scr.tile([128, 1], F32, tag="st")
                    nc.vector.tensor_reduce(
                        st, phsb[:, G, :], axis=mybir.AxisListType.X,
                        op=mybir.AluOpType.max, apply_absolute_value=True,
                    )
                    nc.vector.tensor_scalar_add(hw[:, G : G + 1], st, 1e-6)
                pt = projps.tile([128, 512], F32, tag="pp")
                for k in range(KC):
                    nc.tensor.matmul(
                        pt, lhsT=thwt_sb[:, k, :], rhs=x1t_sb[:, k, sl],
                        start=(k == 0), stop=(k == KC - 1),
                    )
                nc.vector.tensor_scalar_add(thT_sb[:, sl], pt, thb_sb)
            nc.vector.reciprocal(ihw, hw)
            for G in range(ng):
                nc.vector.tensor_scalar_mul(
                    phi16[:, G, :], phsb[:, G, :], ihw[:, G : G + 1]
                )
            # hw broadcast: [128, ng] -> (DRAM round trip) -> [1, bc]
            # -> ones-outer matmul -> [128, bc]
            nc.sync.dma_start(hwdram[:].rearrange("(p g) -> p g", p=128), hw)
            nc.sync.dma_start(
                hwrow.rearrange("o (g p) -> o g p", p=128),
                hwdram[:].rearrange("(p g) -> g p", p=128),
            )
            for ch in range(nch):
                sl = slice(ch * 512, (ch + 1) * 512)
                xg = x0in.tile([128, KC, 512], F16, tag="xg")
                nc.sync.dma_start(
                    xg, x0t[:, sl].rearrange("(k p) b -> p k b", p=128)
                )
                hb = projps.tile([128, 512], F32, tag="pp")
                nc.tensor.matmul(hb, lhsT=ones_row32, rhs=hwrow[:, sl])
                nc.vector.tensor_mul(that32[:, sl], thT_sb[:, sl], hb)
                gp = projps.tile([128, 512], F32, tag="pp")
                for k in range(KC):
                    nc.tensor.matmul(
                        gp, lhsT=gwt_sb[:, k, :], rhs=xg[:, k, :],
                        start=(k == 0), stop=(k == KC - 1),
                    )
                nc.vector.tensor_scalar_add(g16[:, sl], gp, gb_sb)

        # ================= P2: grid =================
        with tc.tile_pool(name="gridpsum", bufs=1, space="PSUM") as gridpsum:
            numps = gridpsum.tile([128, bc // 4], F32)
            denps = gridpsum.tile([128, bc // 4], F32)
            with (
                tc.tile_pool(name="epool", bufs=2) as epool,
                tc.tile_pool(name="gepool", bufs=2) as gepool,
            ):
                for li in range(L):
                    E = epool.tile([128, bc], BF16, tag="e")
                    nc.scalar.activation(
                        E, that32, mybir.ActivationFunctionType.Exp,
                        scale=float(nodes[li]),
                    )
                    gE = gepool.tile([128, bc], BF16, tag="ge")
                    nc.vector.tensor_mul(gE, g16, E)
                    Ev = E.rearrange("p (c r) -> p c r", r=4)
                    gEv = gE.rearrange("p (c r) -> p c r", r=4)
                    for r in range(4):
                        j = 32 * (r % 2) + li  # column within the 64-block
                        oh = id127[:, 63 - j : 127 - j]  # ones in column j
                        base = 64 * (r // 2)
                        st = li == 0 and r % 2 == 0
                        sp = li == L - 1 and r % 2 == 1
                        nc.tensor.matmul(
                            denps[base : base + 64, :], lhsT=oh,
                            rhs=Ev[:, :, r], start=st, stop=sp,
                        )
                        nc.tensor.matmul(
                            numps[base : base + 64, :], lhsT=oh,
                            rhs=gEv[:, :, r], start=st, stop=sp,
                        )

            # ================= P2b: ygrid + fit =================
            with tc.tile_pool(name="cps", bufs=4, space="PSUM") as cpsp:
                nc.vector.tensor_scalar_add(dpre, denps, 1e-20)
                nc.vector.reciprocal(rden, dpre)
                nc.vector.tensor_mul(ygrid, numps, rden)
                csv = csb.rearrange("p (q r) -> p q r", r=4)
                for r in range(4):
                    cp = cpsp.tile([128, bc // 4], F32, tag="cp")
                    nc.tensor.matmul(cp, lhsT=fm_sb[:, r, :], rhs=ygrid)
                    nc.vector.tensor_copy(csv[:, :, r], cp)

        # ================= P3: eval + final =================
        NQB = 8  # quads per batch
        nbatch = nq // NQB
        with (
            tc.tile_pool(name="argps", bufs=2, space="PSUM") as argps,
            tc.tile_pool(name="e2pool", bufs=3) as e2pool,
            tc.tile_pool(name="ypsp", bufs=2, space="PSUM") as ypsp,
            tc.tile_pool(name="finps", bufs=2, space="PSUM") as finps,
            tc.tile_pool(name="resid", bufs=4) as resid,
            tc.tile_pool(name="osb", bufs=3) as osb,
        ):
            xr_tiles = [None] * ng
            yps_tiles = [None] * ng

            def emit_realign(GP):
                # realign two groups (2*GP, 2*GP+1) into one buffer
                qb = qbufs[GP % 2]
                engs = (nc.gpsimd, nc.sync, nc.scalar, nc.gpsimd)
                for p in range(4):
                    engs[p].dma_start(
                        qb[p : p + 1, :, :, :],
                        phi16[p : 128 : 4, 2 * GP : 2 * GP + 2, :],
                    )

            def emit_xr(G):
                xr_tiles[G] = resid.tile([128, C], F16, tag="xr", name="xrt")
                nc.sync.dma_start(xr_tiles[G], x0r[G * 128 : (G + 1) * 128, :])

            def emit_final(G):
                yv = y16[:, G * 128 : (G + 1) * 128]
                ot = osb.tile([128, C], F32, tag="ot")
                for h in range(2):
                    fp = finps.tile([128, 512], F32, tag="fp")
                    nc.tensor.matmul(
                        fp, lhsT=yv, rhs=wwt_sb[:, h * 512 : (h + 1) * 512]
                    )
                    nc.vector.tensor_add(
                        ot[:, h * 512 : (h + 1) * 512], fp,
                        xr_tiles[G][:, h * 512 : (h + 1) * 512],
                    )
                nc.sync.dma_start(out[G * 128 : (G + 1) * 128, :], ot)

            emit_realign(0)
            emit_xr(0)
            emit_xr(1)
            for t in range(nbatch):
                G, tq = divmod(t * NQB, 32)  # group, quad offset within group
                if tq == 0:
                    if G % 2 == 0 and G // 2 + 1 < ng // 2:
                        emit_realign(G // 2 + 1)
                    if G + 2 < ng:
                        emit_xr(G + 2)
                    yps_tiles[G] = ypsp.tile([128, 128], F32, tag="yp", name="ypt")
                qb = qbufs[(G // 2) % 2]
                gsub = G % 2
                ap = argps.tile([128, NQB * 128], F32, tag="ap")
                for h in range(2):
                    nc.tensor.matmul(
                        ap[:, h * 512 : (h + 1) * 512],
                        lhsT=coeft_sb,
                        rhs=qb[:, tq + h * 4 : tq + (h + 1) * 4, gsub, :],
                    )
                e2 = e2pool.tile([128, NQB * 128], BF16, tag="e2")
                nc.scalar.activation(e2, ap, mybir.ActivationFunctionType.Tanh)
                for j in range(NQB):
                    q = t * NQB + j  # global quad
                    nc.tensor.matmul(
                        yps_tiles[G][:, (tq + j) * 4 : (tq + j) * 4 + 4],
                        lhsT=e2[:, j * 128 : (j + 1) * 128],
                        rhs=csb[:, 4 * q : 4 * q + 4],
                    )
                if tq + NQB == 32:  # group complete
                    nc.vector.tensor_copy(
                        y16[:, G * 128 : (G + 1) * 128], yps_tiles[G]
                    )
                    emit_final(G)

    nc.compile()
    return nc


_BASS_CACHE = {}


def _get_bass(bc):
    if bc not in _BASS_CACHE:
        _BASS_CACHE[bc] = build_bass(bc)
    return _BASS_CACHE[bc]


def make_core_inputs(x0, x1, g_w, g_b, theta_w, theta_b, phi_w, phi_b, W_w, W_b,
                     bc=None, ncores=NCORES):
    import ml_dtypes

    n = x0.shape[0] if bc is None else bc * ncores
    bc = n // ncores

    x0 = np.asarray(x0, np.float32)[:n]
    x1 = np.asarray(x1, np.float32)[:n]
    x1t = np.ascontiguousarray(x1.T.astype(np.float16))
    x0t = np.ascontiguousarray(x0.T.astype(np.float16))
    x0r = x0 if not np.any(W_b) else (x0 + np.asarray(W_b, np.float32)[None, :])
    x0r = np.ascontiguousarray(x0r, dtype=np.float16)

    thwt = np.ascontiguousarray(np.asarray(theta_w).T.astype(np.float16))
    phwt = np.ascontiguousarray(np.asarray(phi_w).T.astype(np.float16))
    gwt = np.ascontiguousarray(np.asarray(g_w).T.astype(np.float16))
    wwt = np.ascontiguousarray(np.asarray(W_w).T.astype(ml_dtypes.bfloat16))
    thbc = np.ascontiguousarray(np.asarray(theta_b, np.float32).reshape(D, 1))
    gbc = np.ascontiguousarray(np.asarray(g_b, np.float32).reshape(D, 1))
    phbc = np.ascontiguousarray(
        np.tile(np.asarray(phi_b, np.float32)[None, :], (128, 1))
    )

    nodes, cents, betas = _basis_params()
    F = _fit_matrix()  # [NB, L]
    # fmat[r]: [128, 128] lhsT, [(r2,l), (r3,m)] = F[m, l] iff r2==r3==r
    fmat = np.zeros((4, 128, 128), np.float32)
    for r in range(4):
        fmat[r, 32 * r : 32 * r + L, 32 * r : 32 * r + NB] = F.T
    fmat = np.ascontiguousarray(fmat.reshape(4 * 128, 128).astype(np.float32))
    # coeft [5, 128]: col (32r+l): row r = betas[l], row 4 = -betas[l]*cents[l]
    coeft = np.zeros((5, 128), np.float32)
    for r in range(4):
        for l in range(NB):
            coeft[r, 32 * r + l] = betas[l]
            coeft[4, 32 * r + l] = -betas[l] * cents[l]
    # note: col (32r+l) row 4 shared across r -> -betas*cents placed per col
    coeft = np.ascontiguousarray(coeft.astype(np.float16))

    in_maps = []
    for c in range(ncores):
        sl = slice(c * bc, (c + 1) * bc)
        in_maps.append(
            {
                "x1t": np.ascontiguousarray(x1t[:, sl]),
                "x0t": np.ascontiguousarray(x0t[:, sl]),
                "x0r": np.ascontiguousarray(x0r[sl]),
                "thwt": thwt,
                "phwt": phwt,
                "gwt": gwt,
                "wwt": wwt,
                "thb": thbc,
                "gb": gbc,
                "phb": phbc,
                "fmat": fmat,
                "coeft": coeft,
            }
        )
    return in_maps, bc


def kernel(x0, x1, g_w, g_b, theta_w, theta_b, phi_w, phi_b, W_w, W_b):
    from concourse.bass_utils import run_bass_kernel_spmd

    in_maps, bc = make_core_inputs(
        x0, x1, g_w, g_b, theta_w, theta_b, phi_w, phi_b, W_w, W_b
    )
    nc = _get_bass(bc)
    res = run_bass_kernel_spmd(nc, in_maps, core_ids=list(range(NCORES)))
    outs = [r["out"] for r in res.results]
    return np.ascontiguousarray(np.concatenate(outs, axis=0), dtype=np.float32)
